# revision 24
# baseline (speedup 1.0000x reference)
"""Trainium2 Bass kernel for nn_CAML_53240414601378.

Embedding lookup -> Conv1d(k=4, pad=2) -> tanh -> per-label attention
pooling -> logits. Data-parallel over batch across 8 NeuronCores
(4 batches per core); small params replicated.

Key structure:
- Embedding gather: single SWDGE queue, ~2k-idx chunks chained in issue
  order (the critical fix vs v1: the dependency edge direction -- v1's
  chain was accidentally REVERSED, so chunks gathered backwards and all
  compute serialized after the full chain). Multi-queue SWDGE measured
  ~4x faster but corrupts data (concurrent desc-gen ucode race) -- do
  not use. The gather chain (~130-155us at ~8-10ns/idx, run-to-run
  power-state dependent) is the critical path; the PE (~135us) runs
  just beneath it.
- Chunk boundaries sit at 512j+128-style offsets: a boundary at a conv
  tile edge (512j) splits two tiles' windows and gates tile j-1 on the
  next chunk; off-edge boundaries split one tile and gate none.
- Last batch gathers its final 128 cols FIRST, then bulk, then a
  shrinking cascade, so little conv work stays gated on the chain end.
- dma_gather(transpose=True) pulls bf16 embedding rows straight into
  (e%128, e//128, s) layout -- no on-chip transpose.
- conv1d(k=4) = 4 shifted bf16 matmuls x 2 E-chunks x 2 F-chunks into
  PSUM; boundaries handled with shrunken-N matmuls (no padding).
- scores = U_w @ H and t = final_w @ H come from ONE matmul per
  (F-chunk, seq-tile) with a combined stationary operand (U_w -> psum
  rows 0..49, final_w -> rows 64..113). The scores matmul for tile j
  issues AFTER conv of tile j+1 so the tanh latency of tile j never
  stalls the PE stream.
- online softmax: per seq-tile partial (-max, Z, num) computed straight
  off the PSUM tile; tiny (50, 9) combine at the end. logits =
  sum_s alpha * t + final_b -- the (B,L,F) intermediate of the
  reference is never materialized.
"""

import numpy as np
import ml_dtypes

import concourse.bass as bass
import concourse.tile as tile
from concourse.tile import add_dep_helper
from concourse import bacc, mybir
from concourse.bass_utils import run_bass_kernel_spmd

B, S = 32, 4096
VOCAB, E, F, L = 30522, 256, 256, 50
SO = S + 1  # conv output length (4097)
BF16 = mybir.dt.bfloat16
FP32 = mybir.dt.float32
N_CORES = 8
BPC = B // N_CORES  # batches per core
NT, TN = 8, 512  # full seq tiles covering t in [0, 4096)
NJ = NT + 1      # score tiles (8x512 + 1)

# v5 gather: multi-queue SWDGE. The dma_gather ucode only uses the
# gpsimd cpu pair cpu_id/2 == queue_num, so single-queue desc-gen
# (8.4ns/idx, 137us for 16384 idx) left 6 of 8 DSPs idle and starved
# the PE ~3us per batch. Chunks round-robin over the 4 SWDGE queues
# with prepare_only desc-gen (runs concurrently across cpu pairs,
# verified bit-exact) and a serialized trigger chain: concurrent
# transpose-gather TRANSFERS would interleave 256B spray packets
# through the single per-core SBUF xbar and corrupt data, so each
# trigger waits the previous chunk's DMA-completion sem (+16/chunk).
# Emission is strict rounds of [<=4 preps, one per queue][triggers]:
# count=1 triggers + manual prep sems deadlock on HW, and count=None
# capture requires exactly one pending prep per queue per trigger.
# Consumers are sync-gated COARSELY: one edge per conv (tile,fc) group
# on its last chunk's completion anchor (per-matmul edges -> 210us
# schedule blowup; no edges -> NaN, the tile framework mis-wires
# deferred-trigger prep deps).
# Chunk plan: batch 0 uses small chunks (640/768) so the PE ramps
# right behind the first triggers; later batches use 1024-chunks
# (fewer IncSwdgeSem/trigger overheads). Supply ~16us/batch < PE burn
# ~31us/batch, so the PE streams gaplessly once started.
# b0: small head chunks for an early PE start, and exactly 4 chunks so
# the whole batch fires in trigger round 0 (a 5th chunk would slip to
# round 1 and starve PE tiles 6-7, the 1.1us gap seen in v6). Later
# batches use few, large, 512-aligned chunks: every chunk boundary
# splits conv k-windows into extra piece-matmuls (+8 mms per mid-tile
# boundary, +6 per tile-edge boundary), so fewer/edge-aligned
# boundaries cut ~84 tiny matmuls vs the all-1024 plan. b3 is a single
# 4096-chunk (gen 37us on its cpu pair, lands ~87us; PE only reaches
# b3 at ~122us).
PLANS = [
    [(0, 640), (640, 640), (1280, 1280), (2560, 1536)],
    [(0, 1024), (1024, 1024), (2048, 2048)],
    [(0, 2048), (2048, 2048)],
    [(0, 4096)],
]
NQ = 4

_cache = {}


def _conv_mms(t0, n):
    """Conv matmul pieces for output cols [t0, t0+n): (k, lo, hi, off),
    full-width first so start=True covers the whole psum range."""
    shifts = []
    for k in range(4):
        lo = max(0, t0 + k - 2)
        hi = min(S, t0 + k - 2 + n)
        shifts.append((k, lo, hi, lo - (t0 + k - 2)))
    shifts.sort(key=lambda s: -(s[2] - s[1]))
    return shifts


def build_nc():
    nc = bacc.Bacc("TRN2", target_bir_lowering=False, debug=False,
                   num_devices=N_CORES, num_swdge_queues=NQ)

    emb_ap = nc.dram_tensor("emb", (VOCAB, E), BF16, kind="ExternalInput").ap()
    idx_ap = nc.dram_tensor("idx", (128, BPC * S // 16), mybir.dt.int16,
                            kind="ExternalInput").ap()
    w_ap = nc.dram_tensor("wconv", (128, 16, 128), BF16,
                          kind="ExternalInput").ap()
    uw_ap = nc.dram_tensor("uwfw", (128, 2, 114), BF16,
                           kind="ExternalInput").ap()
    cb_ap = nc.dram_tensor("cbias", (128, 2), FP32, kind="ExternalInput").ap()
    fb_ap = nc.dram_tensor("fbias", (L, 1), FP32, kind="ExternalInput").ap()
    out_ap = nc.dram_tensor("out", (L, BPC), FP32, kind="ExternalOutput").ap()

    with tile.TileContext(nc) as tc:
        with (
            tc.tile_pool(name="const", bufs=1) as const,
            tc.tile_pool(name="xh", bufs=1) as xh,     # gather chunk tiles
            tc.tile_pool(name="hp", bufs=2) as hp,
            tc.tile_pool(name="ep", bufs=3) as ep,     # exp scratch tiles
            tc.tile_pool(name="pp", bufs=2) as pp,     # per-batch partials
            tc.tile_pool(name="small", bufs=8) as small,
            tc.tile_pool(name="psum", bufs=2, space="PSUM") as psum,
            tc.tile_pool(name="psum_st", bufs=4, space="PSUM") as psum_st,
        ):
            IPB = S // 16  # idx columns per batch

            # ---- constants; idx first (sync/HWDGE so gpsimd queue stays
            # empty until the first gather -> library load starts at ~0) ----
            idx_sb = const.tile([128, BPC * S // 16], mybir.dt.int16)
            nc.sync.dma_start(idx_sb[:, 0:IPB], idx_ap[:, 0:IPB])
            nc.sync.dma_start(idx_sb[:, IPB:], idx_ap[:, IPB:])
            w_sb = const.tile([128, 16, 128], BF16)
            nc.sync.dma_start(w_sb[:], w_ap[:])
            uw_sb = const.tile([128, 2, 114], BF16)
            nc.sync.dma_start(uw_sb[:], uw_ap[:])
            cb_sb = const.tile([128, 2], FP32)
            nc.sync.dma_start(cb_sb[:], cb_ap[:])
            fb_sb = const.tile([L, 1], FP32)
            nc.sync.dma_start(fb_sb[:], fb_ap[:])
            out_sb = const.tile([L, BPC], FP32)

            # ---- gathers: rounds of <=4 concurrent preps (one per
            # SWDGE queue) + xbar-serialized triggers ----
            chunks = [(b, g0, cs) for b in range(BPC)
                      for (g0, cs) in PLANS[b]]
            NCHUNKS = len(chunks)
            dma_sems = [nc.alloc_semaphore(f"gdma{q}") for q in range(NQ)]
            all_segs = [[] for _ in range(BPC)]
            ready = [None] * NCHUNKS  # chunk-completion wait anchors
            last_prep = [None] * NQ
            prev_t = None
            for r0 in range(0, NCHUNKS, NQ):
                rng = range(r0, min(r0 + NQ, NCHUNKS))
                rps = {}
                for g in rng:
                    q = g % NQ
                    b, g0, cs = chunks[g]
                    ci = len(all_segs[b])
                    xt = xh.tile([128, 2, cs], BF16, tag=f"b{b}c{ci}",
                                 name=f"x_b{b}c{ci}")
                    gi = nc.gpsimd.dma_gather(
                        out_ap=xt[:], in_ap=emb_ap[:],
                        idxs_ap=idx_sb[:, b * IPB + g0 // 16:
                                       b * IPB + (g0 + cs) // 16],
                        num_idxs=cs, num_idxs_reg=cs, elem_size=E,
                        transpose=True, single_packet=False,
                        prepare_only=True, sem=dma_sems[q], queue_num=q)
                    if last_prep[q] is not None:
                        add_dep_helper(gi.ins, last_prep[q].ins, False,
                                       "per-queue prep order")
                    last_prep[q] = gi
                    rps[q] = gi
                    all_segs[b].append((xt, g0, g0 + cs, g))
                for g in rng:
                    q = g % NQ
                    if g > 0:
                        w = nc.gpsimd.wait_ge(dma_sems[(g - 1) % NQ],
                                              16 * ((g - 1) // NQ + 1))
                        add_dep_helper(w.ins, prev_t.ins, False,
                                       "xbar trigger chain")
                        add_dep_helper(w.ins, rps[q].ins, False,
                                       "after round prep")
                        ready[g - 1] = w
                        prev_t = w
                    tr = nc.gpsimd.trigger_dma(count=None, queue_num=q)
                    if prev_t is not None:
                        add_dep_helper(tr.ins, prev_t.ins, False,
                                       "trigger order")
                    prev_t = tr
            wlast = nc.gpsimd.wait_ge(dma_sems[(NCHUNKS - 1) % NQ],
                                      16 * ((NCHUNKS - 1) // NQ + 1))
            add_dep_helper(wlast.ins, prev_t.ins, False, "final chunk wait")
            ready[NCHUNKS - 1] = wlast

            for b in range(BPC):
                segs = all_segs[b]

                def rhs_pieces(lo, hi):
                    out = []
                    for (xt, c0, c1, g) in segs:
                        a, bnd = max(lo, c0), min(hi, c1)
                        if a < bnd:
                            out.append((xt, a - c0, bnd - c0, a - lo, g))
                    return out

                H = hp.tile([128, 2, SO], BF16, tag="H")
                nmx = pp.tile([L, NJ], FP32, tag="nmx")  # -max per tile
                zp = pp.tile([L, NJ], FP32, tag="zp")    # partial Z
                np_ = pp.tile([L, NJ], FP32, tag="np")   # partial num

                def score_tile(j, n):
                    """Combined scores/t matmul for H cols [j*TN, +n) and
                    the online-softmax partials for that tile."""
                    t0 = j * TN
                    pst = psum_st.tile([114, TN], FP32, tag="st")
                    for fc in range(2):
                        nc.tensor.matmul(
                            pst[:, 0:n], uw_sb[:, fc, :], H[:, fc, t0:t0 + n],
                            start=(fc == 0), stop=(fc == 1),
                        )
                    nc.vector.reduce_max(nmx[:, j:j + 1], pst[0:L, 0:n],
                                         axis=mybir.AxisListType.X,
                                         negate=True)
                    e_sb = ep.tile([L, TN], FP32, tag="e")
                    nc.scalar.activation(
                        e_sb[:, 0:n], pst[0:L, 0:n],
                        mybir.ActivationFunctionType.Exp,
                        bias=nmx[:, j:j + 1], accum_out=zp[:, j:j + 1],
                    )
                    nc.vector.tensor_mul(e_sb[:, 0:n], e_sb[:, 0:n],
                                         pst[64:64 + L, 0:n])
                    nc.vector.reduce_sum(np_[:, j:j + 1], e_sb[:, 0:n],
                                         axis=mybir.AxisListType.X)

                # ---- conv1d(k=4) + bias + tanh per seq tile; the scores
                # matmul for tile j issues after conv of tile j+1 ----
                for j in range(NT):
                    t0 = j * TN
                    for fc in range(2):
                        ph = psum.tile([128, TN], FP32, tag=f"h{fc}")
                        mms = []
                        for (k, lo, hi, off) in _conv_mms(t0, TN):
                            for (xt, a, bnd, rel, g) in rhs_pieces(lo, hi):
                                for ec in range(2):
                                    mms.append((k, ec, xt, a, bnd, off + rel,
                                                g))
                        gmax = max(m[6] for m in mms)
                        for i, (k, ec, xt, a, bnd, off, g) in enumerate(mms):
                            mi = nc.tensor.matmul(
                                ph[:, off:off + (bnd - a)],
                                w_sb[:, k * 4 + ec * 2 + fc, :],
                                xt[:, ec, a:bnd],
                                start=(i == 0), stop=(i == len(mms) - 1),
                            )
                            if i == 0:
                                # coarse gate: whole group (in-order on
                                # PE) on the tile's LAST chunk landing
                                add_dep_helper(mi.ins, ready[gmax].ins,
                                               True, "tile chunks ready")
                        nc.scalar.activation(
                            H[:, fc, t0:t0 + TN], ph[:],
                            mybir.ActivationFunctionType.Tanh,
                            bias=cb_sb[:, fc:fc + 1],
                        )
                    if j > 0:
                        score_tile(j - 1, TN)

                # last output column t = 4096 (x cols 4094..4095)
                (lseg, lg0, _, lg) = next(
                    s for s in segs if s[1] <= 4094 < s[2])
                for fc in range(2):
                    ph9 = psum.tile([128, 1], FP32, tag=f"h{fc}")
                    i = 0
                    for k in range(2):
                        for ec in range(2):
                            mi = nc.tensor.matmul(
                                ph9[:, 0:1],
                                w_sb[:, k * 4 + ec * 2 + fc, :],
                                lseg[:, ec, 4094 - lg0 + k:4095 - lg0 + k],
                                start=(i == 0), stop=(i == 3),
                            )
                            if i == 0:
                                add_dep_helper(mi.ins, ready[lg].ins, True,
                                               "lastcol chunk ready")
                            i += 1
                    nc.scalar.activation(
                        H[:, fc, S:SO], ph9[:],
                        mybir.ActivationFunctionType.Tanh,
                        bias=cb_sb[:, fc:fc + 1],
                    )
                # tile 8 (the t=4096 column) first: its partial chain
                # (exp/mul/reduce) then overlaps score_tile(7)'s matmuls
                # instead of sitting on the critical tail after them
                score_tile(NT, 1)
                score_tile(NT - 1, TN)

                # ---- combine partials -> logits ----
                nm = small.tile([L, 1], FP32, tag="nm")  # -(global max)
                nc.vector.reduce_max(nm[:], nmx[:], axis=mybir.AxisListType.X,
                                     op=mybir.AluOpType.min)
                wj = small.tile([L, NJ], FP32, tag="wj")
                nc.scalar.activation(
                    wj[:], nmx[:], mybir.ActivationFunctionType.Exp,
                    bias=nm[:], scale=-1.0,
                )
                wz = small.tile([L, NJ], FP32, tag="wz")
                nc.vector.tensor_mul(wz[:], wj[:], zp[:])
                zsum = small.tile([L, 1], FP32, tag="zsum")
                nc.vector.reduce_sum(zsum[:], wz[:], axis=mybir.AxisListType.X)
                nc.vector.tensor_mul(wj[:], wj[:], np_[:])
                nsum = small.tile([L, 1], FP32, tag="nsum")
                nc.vector.reduce_sum(nsum[:], wj[:], axis=mybir.AxisListType.X)
                zr = small.tile([L, 1], FP32, tag="zr")
                nc.vector.reciprocal(zr[:], zsum[:])
                # logits = nsum/zsum + fb fused into one scalar op
                nc.scalar.activation(
                    out_sb[:, b:b + 1], nsum[:],
                    mybir.ActivationFunctionType.Identity,
                    bias=fb_sb[:], scale=zr[:],
                )

            nc.sync.dma_start(out_ap[:], out_sb[:])

    nc.compile()
    return nc


def _prep_shared(emb_table, conv_w, conv_b, U_w, final_w, final_b):
    emb_bf = np.ascontiguousarray(emb_table.astype(ml_dtypes.bfloat16))

    # wconv[e_lo, k*4 + ec*2 + fc, f_lo] = conv_w[fc*128+f, ec*128+e, k]
    W = np.empty((128, 16, 128), np.float32)
    for k in range(4):
        for ec in range(2):
            for fc in range(2):
                W[:, k * 4 + ec * 2 + fc, :] = conv_w[
                    fc * 128:(fc + 1) * 128, ec * 128:(ec + 1) * 128, k].T
    W = np.ascontiguousarray(W.astype(ml_dtypes.bfloat16))

    # uwfw[f_lo, fc, j]: j<50 -> U_w[j, fc*128+f_lo];
    # j in [64,114) -> final_w[j-64, fc*128+f_lo]; rest zero
    UW = np.zeros((128, 2, 114), np.float32)
    UW[:, :, 0:L] = U_w.T.reshape(2, 128, L).transpose(1, 0, 2)
    UW[:, :, 64:64 + L] = final_w.T.reshape(2, 128, L).transpose(1, 0, 2)
    UW = np.ascontiguousarray(UW.astype(ml_dtypes.bfloat16))

    CB = np.ascontiguousarray(conv_b.reshape(2, 128).T.astype(np.float32))
    FB = np.ascontiguousarray(final_b.reshape(L, 1).astype(np.float32))
    return emb_bf, W, UW, CB, FB


def kernel(input_ids, emb_table, conv_w, conv_b, U_w, final_w, final_b):
    import os
    ids = np.asarray(input_ids)
    emb_table = np.asarray(emb_table, dtype=np.float32)
    conv_w = np.asarray(conv_w, dtype=np.float32)
    conv_b = np.asarray(conv_b, dtype=np.float32)
    U_w = np.asarray(U_w, dtype=np.float32)
    final_w = np.asarray(final_w, dtype=np.float32)
    final_b = np.asarray(final_b, dtype=np.float32)

    if "nc" not in _cache:
        _cache["nc"] = build_nc()
    nc = _cache["nc"]

    emb_bf, W, UW, CB, FB = _prep_shared(
        emb_table, conv_w, conv_b, U_w, final_w, final_b)

    ids16 = ids.astype(np.int16)  # vocab 30522 < 2**15
    in_maps = []
    for c in range(N_CORES):
        cid = ids16[c * BPC:(c + 1) * BPC]  # (BPC, S)
        # position i -> [i % 16, i // 16], batches along axis 1; the
        # 16-row block is replicated to all 8 gpsimd cores (128 rows)
        blk = np.concatenate(
            [cid[b].reshape(S // 16, 16).T for b in range(BPC)], axis=1)
        idx = np.tile(blk, (8, 1))
        in_maps.append({
            "emb": emb_bf, "idx": np.ascontiguousarray(idx),
            "wconv": W, "uwfw": UW, "cbias": CB, "fbias": FB,
        })

    trace = bool(int(os.environ.get("KERNEL_TRACE", "0")))
    try:
        res = run_bass_kernel_spmd(nc, in_maps,
                                   core_ids=list(range(N_CORES)),
                                   trace=trace)
    except ModuleNotFoundError:
        # tracing hook unavailable in this environment; run untraced
        res = run_bass_kernel_spmd(nc, in_maps,
                                   core_ids=list(range(N_CORES)),
                                   trace=False)
    _cache["last_result"] = res

    out = np.concatenate(
        [res.results[c]["out"].T for c in range(N_CORES)], axis=0)
    return np.ascontiguousarray(out.astype(np.float32))


# revision 26
# speedup vs baseline: 1.0202x; 1.0202x over previous
"""Trainium2 Bass kernel for nn_CAML_53240414601378.

Embedding lookup -> Conv1d(k=4, pad=2) -> tanh -> per-label attention
pooling -> logits. Data-parallel over batch across 8 NeuronCores
(4 batches per core); small params replicated.

Key structure:
- Embedding gather: single SWDGE queue, ~2k-idx chunks chained in issue
  order (the critical fix vs v1: the dependency edge direction -- v1's
  chain was accidentally REVERSED, so chunks gathered backwards and all
  compute serialized after the full chain). Multi-queue SWDGE measured
  ~4x faster but corrupts data (concurrent desc-gen ucode race) -- do
  not use. The gather chain (~130-155us at ~8-10ns/idx, run-to-run
  power-state dependent) is the critical path; the PE (~135us) runs
  just beneath it.
- Chunk boundaries sit at 512j+128-style offsets: a boundary at a conv
  tile edge (512j) splits two tiles' windows and gates tile j-1 on the
  next chunk; off-edge boundaries split one tile and gate none.
- Last batch gathers its final 128 cols FIRST, then bulk, then a
  shrinking cascade, so little conv work stays gated on the chain end.
- dma_gather(transpose=True) pulls bf16 embedding rows straight into
  (e%128, e//128, s) layout -- no on-chip transpose.
- conv1d(k=4) = 4 shifted bf16 matmuls x 2 E-chunks x 2 F-chunks into
  PSUM; boundaries handled with shrunken-N matmuls (no padding).
- scores = U_w @ H and t = final_w @ H come from ONE matmul per
  (F-chunk, seq-tile) with a combined stationary operand (U_w -> psum
  rows 0..49, final_w -> rows 64..113). The scores matmul for tile j
  issues AFTER conv of tile j+1 so the tanh latency of tile j never
  stalls the PE stream.
- online softmax: per seq-tile partial (-max, Z, num) computed straight
  off the PSUM tile; tiny (50, 9) combine at the end. logits =
  sum_s alpha * t + final_b -- the (B,L,F) intermediate of the
  reference is never materialized.
"""

import numpy as np
import ml_dtypes

import concourse.bass as bass
import concourse.tile as tile
from concourse.tile import add_dep_helper
from concourse import bacc, mybir
from concourse.bass_utils import run_bass_kernel_spmd

B, S = 32, 4096
VOCAB, E, F, L = 30522, 256, 256, 50
SO = S + 1  # conv output length (4097)
BF16 = mybir.dt.bfloat16
FP32 = mybir.dt.float32
N_CORES = 8
BPC = B // N_CORES  # batches per core
NT, TN = 8, 512  # full seq tiles covering t in [0, 4096)
NJ = NT + 1      # score tiles (8x512 + 1)

# v5 gather: multi-queue SWDGE. The dma_gather ucode only uses the
# gpsimd cpu pair cpu_id/2 == queue_num, so single-queue desc-gen
# (8.4ns/idx, 137us for 16384 idx) left 6 of 8 DSPs idle and starved
# the PE ~3us per batch. Chunks round-robin over the 4 SWDGE queues
# with prepare_only desc-gen (runs concurrently across cpu pairs,
# verified bit-exact) and a serialized trigger chain: concurrent
# transpose-gather TRANSFERS would interleave 256B spray packets
# through the single per-core SBUF xbar and corrupt data, so each
# trigger waits the previous chunk's DMA-completion sem (+16/chunk).
# Emission is strict rounds of [<=4 preps, one per queue][triggers]:
# count=1 triggers + manual prep sems deadlock on HW, and count=None
# capture requires exactly one pending prep per queue per trigger.
# Consumers are sync-gated COARSELY: one edge per conv (tile,fc) group
# on its last chunk's completion anchor (per-matmul edges -> 210us
# schedule blowup; no edges -> NaN, the tile framework mis-wires
# deferred-trigger prep deps).
# Chunk plan: batch 0 uses small chunks (640/768) so the PE ramps
# right behind the first triggers; later batches use 1024-chunks
# (fewer IncSwdgeSem/trigger overheads). Supply ~16us/batch < PE burn
# ~31us/batch, so the PE streams gaplessly once started.
# b0: small head chunks for an early PE start, and exactly 4 chunks so
# the whole batch fires in trigger round 0 (a 5th chunk would slip to
# round 1 and starve PE tiles 6-7, the 1.1us gap seen in v6). Later
# batches use few, large, 512-aligned chunks: every chunk boundary
# splits conv k-windows into extra piece-matmuls (+8 mms per mid-tile
# boundary, +6 per tile-edge boundary), so fewer/edge-aligned
# boundaries cut ~84 tiny matmuls vs the all-1024 plan. b3 is a single
# 4096-chunk (gen 37us on its cpu pair, lands ~87us; PE only reaches
# b3 at ~122us).
PLANS = [
    [(0, 640), (640, 640), (1280, 1280), (2560, 1536)],
    [(0, 2048), (2048, 2048)],
    [(0, 4096)],
    [(0, 4096)],
]
NQ = 4

_cache = {}


def _conv_mms(t0, n):
    """Conv matmul pieces for output cols [t0, t0+n): (k, lo, hi, off),
    full-width first so start=True covers the whole psum range."""
    shifts = []
    for k in range(4):
        lo = max(0, t0 + k - 2)
        hi = min(S, t0 + k - 2 + n)
        shifts.append((k, lo, hi, lo - (t0 + k - 2)))
    shifts.sort(key=lambda s: -(s[2] - s[1]))
    return shifts


def build_nc():
    nc = bacc.Bacc("TRN2", target_bir_lowering=False, debug=False,
                   num_devices=N_CORES, num_swdge_queues=NQ)

    emb_ap = nc.dram_tensor("emb", (VOCAB, E), BF16, kind="ExternalInput").ap()
    idx_ap = nc.dram_tensor("idx", (128, BPC * S // 16), mybir.dt.int16,
                            kind="ExternalInput").ap()
    w_ap = nc.dram_tensor("wconv", (128, 16, 128), BF16,
                          kind="ExternalInput").ap()
    uw_ap = nc.dram_tensor("uwfw", (128, 2, 114), BF16,
                           kind="ExternalInput").ap()
    cb_ap = nc.dram_tensor("cbias", (128, 2), FP32, kind="ExternalInput").ap()
    fb_ap = nc.dram_tensor("fbias", (L, 1), FP32, kind="ExternalInput").ap()
    out_ap = nc.dram_tensor("out", (L, BPC), FP32, kind="ExternalOutput").ap()

    with tile.TileContext(nc) as tc:
        with (
            tc.tile_pool(name="const", bufs=1) as const,
            tc.tile_pool(name="xh", bufs=1) as xh,     # gather chunk tiles
            tc.tile_pool(name="hp", bufs=2) as hp,
            tc.tile_pool(name="ep", bufs=3) as ep,     # exp scratch tiles
            tc.tile_pool(name="pp", bufs=2) as pp,     # per-batch partials
            tc.tile_pool(name="small", bufs=8) as small,
            tc.tile_pool(name="psum", bufs=2, space="PSUM") as psum,
            tc.tile_pool(name="psum_st", bufs=4, space="PSUM") as psum_st,
        ):
            IPB = S // 16  # idx columns per batch

            # ---- constants; idx first (sync/HWDGE so gpsimd queue stays
            # empty until the first gather -> library load starts at ~0) ----
            idx_sb = const.tile([128, BPC * S // 16], mybir.dt.int16)
            nc.sync.dma_start(idx_sb[:, 0:IPB], idx_ap[:, 0:IPB])
            nc.sync.dma_start(idx_sb[:, IPB:], idx_ap[:, IPB:])
            w_sb = const.tile([128, 16, 128], BF16)
            nc.sync.dma_start(w_sb[:], w_ap[:])
            uw_sb = const.tile([128, 2, 114], BF16)
            nc.sync.dma_start(uw_sb[:], uw_ap[:])
            cb_sb = const.tile([128, 2], FP32)
            nc.sync.dma_start(cb_sb[:], cb_ap[:])
            fb_sb = const.tile([L, 1], FP32)
            nc.sync.dma_start(fb_sb[:], fb_ap[:])
            out_sb = const.tile([L, BPC], FP32)

            # ---- gathers: rounds of <=4 concurrent preps (one per
            # SWDGE queue) + xbar-serialized triggers ----
            chunks = [(b, g0, cs) for b in range(BPC)
                      for (g0, cs) in PLANS[b]]
            NCHUNKS = len(chunks)
            dma_sems = [nc.alloc_semaphore(f"gdma{q}") for q in range(NQ)]
            all_segs = [[] for _ in range(BPC)]
            ready = [None] * NCHUNKS  # chunk-completion wait anchors
            last_prep = [None] * NQ
            prev_t = None
            for r0 in range(0, NCHUNKS, NQ):
                rng = range(r0, min(r0 + NQ, NCHUNKS))
                rps = {}
                for g in rng:
                    q = g % NQ
                    b, g0, cs = chunks[g]
                    ci = len(all_segs[b])
                    xt = xh.tile([128, 2, cs], BF16, tag=f"b{b}c{ci}",
                                 name=f"x_b{b}c{ci}")
                    gi = nc.gpsimd.dma_gather(
                        out_ap=xt[:], in_ap=emb_ap[:],
                        idxs_ap=idx_sb[:, b * IPB + g0 // 16:
                                       b * IPB + (g0 + cs) // 16],
                        num_idxs=cs, num_idxs_reg=cs, elem_size=E,
                        transpose=True, single_packet=False,
                        prepare_only=True, sem=dma_sems[q], queue_num=q)
                    if last_prep[q] is not None:
                        add_dep_helper(gi.ins, last_prep[q].ins, False,
                                       "per-queue prep order")
                    last_prep[q] = gi
                    rps[q] = gi
                    all_segs[b].append((xt, g0, g0 + cs, g))
                for g in rng:
                    q = g % NQ
                    if g > 0:
                        w = nc.gpsimd.wait_ge(dma_sems[(g - 1) % NQ],
                                              16 * ((g - 1) // NQ + 1))
                        add_dep_helper(w.ins, prev_t.ins, False,
                                       "xbar trigger chain")
                        add_dep_helper(w.ins, rps[q].ins, False,
                                       "after round prep")
                        ready[g - 1] = w
                        prev_t = w
                    tr = nc.gpsimd.trigger_dma(count=None, queue_num=q)
                    if prev_t is not None:
                        add_dep_helper(tr.ins, prev_t.ins, False,
                                       "trigger order")
                    prev_t = tr
            wlast = nc.gpsimd.wait_ge(dma_sems[(NCHUNKS - 1) % NQ],
                                      16 * ((NCHUNKS - 1) // NQ + 1))
            add_dep_helper(wlast.ins, prev_t.ins, False, "final chunk wait")
            ready[NCHUNKS - 1] = wlast

            for b in range(BPC):
                segs = all_segs[b]

                def rhs_pieces(lo, hi):
                    out = []
                    for (xt, c0, c1, g) in segs:
                        a, bnd = max(lo, c0), min(hi, c1)
                        if a < bnd:
                            out.append((xt, a - c0, bnd - c0, a - lo, g))
                    return out

                H = hp.tile([128, 2, SO], BF16, tag="H")
                nmx = pp.tile([L, NJ], FP32, tag="nmx")  # -max per tile
                zp = pp.tile([L, NJ], FP32, tag="zp")    # partial Z
                np_ = pp.tile([L, NJ], FP32, tag="np")   # partial num

                def score_tile(j, n):
                    """Combined scores/t matmul for H cols [j*TN, +n) and
                    the online-softmax partials for that tile."""
                    t0 = j * TN
                    pst = psum_st.tile([114, TN], FP32, tag="st")
                    for fc in range(2):
                        nc.tensor.matmul(
                            pst[:, 0:n], uw_sb[:, fc, :], H[:, fc, t0:t0 + n],
                            start=(fc == 0), stop=(fc == 1),
                        )
                    nc.vector.reduce_max(nmx[:, j:j + 1], pst[0:L, 0:n],
                                         axis=mybir.AxisListType.X,
                                         negate=True)
                    e_sb = ep.tile([L, TN], FP32, tag="e")
                    nc.scalar.activation(
                        e_sb[:, 0:n], pst[0:L, 0:n],
                        mybir.ActivationFunctionType.Exp,
                        bias=nmx[:, j:j + 1], accum_out=zp[:, j:j + 1],
                    )
                    nc.vector.tensor_mul(e_sb[:, 0:n], e_sb[:, 0:n],
                                         pst[64:64 + L, 0:n])
                    nc.vector.reduce_sum(np_[:, j:j + 1], e_sb[:, 0:n],
                                         axis=mybir.AxisListType.X)

                # ---- conv1d(k=4) + bias + tanh per seq tile; the scores
                # matmul for tile j issues after conv of tile j+1 ----
                for j in range(NT):
                    t0 = j * TN
                    for fc in range(2):
                        ph = psum.tile([128, TN], FP32, tag=f"h{fc}")
                        mms = []
                        for (k, lo, hi, off) in _conv_mms(t0, TN):
                            for (xt, a, bnd, rel, g) in rhs_pieces(lo, hi):
                                for ec in range(2):
                                    mms.append((k, ec, xt, a, bnd, off + rel,
                                                g))
                        gmax = max(m[6] for m in mms)
                        for i, (k, ec, xt, a, bnd, off, g) in enumerate(mms):
                            mi = nc.tensor.matmul(
                                ph[:, off:off + (bnd - a)],
                                w_sb[:, k * 4 + ec * 2 + fc, :],
                                xt[:, ec, a:bnd],
                                start=(i == 0), stop=(i == len(mms) - 1),
                            )
                            if i == 0:
                                # coarse gate: whole group (in-order on
                                # PE) on the tile's LAST chunk landing
                                add_dep_helper(mi.ins, ready[gmax].ins,
                                               True, "tile chunks ready")
                        nc.scalar.activation(
                            H[:, fc, t0:t0 + TN], ph[:],
                            mybir.ActivationFunctionType.Tanh,
                            bias=cb_sb[:, fc:fc + 1],
                        )
                    if j > 0:
                        score_tile(j - 1, TN)

                # last output column t = 4096 (x cols 4094..4095)
                (lseg, lg0, _, lg) = next(
                    s for s in segs if s[1] <= 4094 < s[2])
                for fc in range(2):
                    ph9 = psum.tile([128, 1], FP32, tag=f"h{fc}")
                    i = 0
                    for k in range(2):
                        for ec in range(2):
                            mi = nc.tensor.matmul(
                                ph9[:, 0:1],
                                w_sb[:, k * 4 + ec * 2 + fc, :],
                                lseg[:, ec, 4094 - lg0 + k:4095 - lg0 + k],
                                start=(i == 0), stop=(i == 3),
                            )
                            if i == 0:
                                add_dep_helper(mi.ins, ready[lg].ins, True,
                                               "lastcol chunk ready")
                            i += 1
                    nc.scalar.activation(
                        H[:, fc, S:SO], ph9[:],
                        mybir.ActivationFunctionType.Tanh,
                        bias=cb_sb[:, fc:fc + 1],
                    )
                score_tile(NT - 1, TN)
                score_tile(NT, 1)

                # ---- combine partials -> logits ----
                nm = small.tile([L, 1], FP32, tag="nm")  # -(global max)
                nc.vector.reduce_max(nm[:], nmx[:], axis=mybir.AxisListType.X,
                                     op=mybir.AluOpType.min)
                wj = small.tile([L, NJ], FP32, tag="wj")
                nc.scalar.activation(
                    wj[:], nmx[:], mybir.ActivationFunctionType.Exp,
                    bias=nm[:], scale=-1.0,
                )
                wz = small.tile([L, NJ], FP32, tag="wz")
                nc.vector.tensor_mul(wz[:], wj[:], zp[:])
                zsum = small.tile([L, 1], FP32, tag="zsum")
                nc.vector.reduce_sum(zsum[:], wz[:], axis=mybir.AxisListType.X)
                nc.vector.tensor_mul(wj[:], wj[:], np_[:])
                nsum = small.tile([L, 1], FP32, tag="nsum")
                nc.vector.reduce_sum(nsum[:], wj[:], axis=mybir.AxisListType.X)
                zr = small.tile([L, 1], FP32, tag="zr")
                nc.vector.reciprocal(zr[:], zsum[:])
                # logits = nsum/zsum + fb fused into one scalar op
                nc.scalar.activation(
                    out_sb[:, b:b + 1], nsum[:],
                    mybir.ActivationFunctionType.Identity,
                    bias=fb_sb[:], scale=zr[:],
                )

            nc.sync.dma_start(out_ap[:], out_sb[:])

    nc.compile()
    return nc


def _prep_shared(emb_table, conv_w, conv_b, U_w, final_w, final_b):
    emb_bf = np.ascontiguousarray(emb_table.astype(ml_dtypes.bfloat16))

    # wconv[e_lo, k*4 + ec*2 + fc, f_lo] = conv_w[fc*128+f, ec*128+e, k]
    W = np.empty((128, 16, 128), np.float32)
    for k in range(4):
        for ec in range(2):
            for fc in range(2):
                W[:, k * 4 + ec * 2 + fc, :] = conv_w[
                    fc * 128:(fc + 1) * 128, ec * 128:(ec + 1) * 128, k].T
    W = np.ascontiguousarray(W.astype(ml_dtypes.bfloat16))

    # uwfw[f_lo, fc, j]: j<50 -> U_w[j, fc*128+f_lo];
    # j in [64,114) -> final_w[j-64, fc*128+f_lo]; rest zero
    UW = np.zeros((128, 2, 114), np.float32)
    UW[:, :, 0:L] = U_w.T.reshape(2, 128, L).transpose(1, 0, 2)
    UW[:, :, 64:64 + L] = final_w.T.reshape(2, 128, L).transpose(1, 0, 2)
    UW = np.ascontiguousarray(UW.astype(ml_dtypes.bfloat16))

    CB = np.ascontiguousarray(conv_b.reshape(2, 128).T.astype(np.float32))
    FB = np.ascontiguousarray(final_b.reshape(L, 1).astype(np.float32))
    return emb_bf, W, UW, CB, FB


def kernel(input_ids, emb_table, conv_w, conv_b, U_w, final_w, final_b):
    import os
    ids = np.asarray(input_ids)
    emb_table = np.asarray(emb_table, dtype=np.float32)
    conv_w = np.asarray(conv_w, dtype=np.float32)
    conv_b = np.asarray(conv_b, dtype=np.float32)
    U_w = np.asarray(U_w, dtype=np.float32)
    final_w = np.asarray(final_w, dtype=np.float32)
    final_b = np.asarray(final_b, dtype=np.float32)

    if "nc" not in _cache:
        _cache["nc"] = build_nc()
    nc = _cache["nc"]

    emb_bf, W, UW, CB, FB = _prep_shared(
        emb_table, conv_w, conv_b, U_w, final_w, final_b)

    ids16 = ids.astype(np.int16)  # vocab 30522 < 2**15
    in_maps = []
    for c in range(N_CORES):
        cid = ids16[c * BPC:(c + 1) * BPC]  # (BPC, S)
        # position i -> [i % 16, i // 16], batches along axis 1; the
        # 16-row block is replicated to all 8 gpsimd cores (128 rows)
        blk = np.concatenate(
            [cid[b].reshape(S // 16, 16).T for b in range(BPC)], axis=1)
        idx = np.tile(blk, (8, 1))
        in_maps.append({
            "emb": emb_bf, "idx": np.ascontiguousarray(idx),
            "wconv": W, "uwfw": UW, "cbias": CB, "fbias": FB,
        })

    trace = bool(int(os.environ.get("KERNEL_TRACE", "0")))
    try:
        res = run_bass_kernel_spmd(nc, in_maps,
                                   core_ids=list(range(N_CORES)),
                                   trace=trace)
    except ModuleNotFoundError:
        # tracing hook unavailable in this environment; run untraced
        res = run_bass_kernel_spmd(nc, in_maps,
                                   core_ids=list(range(N_CORES)),
                                   trace=False)
    _cache["last_result"] = res

    out = np.concatenate(
        [res.results[c]["out"].T for c in range(N_CORES)], axis=0)
    return np.ascontiguousarray(out.astype(np.float32))
